# revision 18
# baseline (speedup 1.0000x reference)
"""AttentionConv2D (3x3 windowed multi-head attention) on 8 TRN2 NeuronCores. v2.

Sharding: data-parallel over batch (B=8 -> 1 image per core), weights replicated.
Per-core layout: channel-major [128 ch, 4096 pix].

Optimizations vs the original baseline (125.5us -> 94.7us, TimelineSim model):
- LN pre-centering z = (x - mu_b) * rstd_b with mu/rstd rows broadcast to all
  128 partitions via one partition-broadcast DMA per quarter (DRAM-bounced),
  replacing per-chunk aug matmuls; stats packed via PE transposes; the last
  stats quarter overlaps the first projections.
- Biases folded into ACT evictions (scalar.add) and Exp(bias=...); V bias
  folded into the output bias host-side (softmax weights sum to 1).
- 2-deep software pipeline interleaving SCORES(c) | AV(c-1) | PROJ(c+2)
  instruction-by-instruction via generators, so in-order engine queues always
  hold ready work; steady state runs at 95-100% engine occupancy; pool depths
  tuned for cross-chunk decoupling.
- GPSIMD cannot access PSUM (hardware rule): evictions live on ACT/DVE, Pool
  runs SBUF-only pk multiplies; 4 of 9 attn*V products read rep PSUM directly.
- Border-only pad memsets, concatenated const blobs (2 DMAs), ACT-table
  preloads, split accumulator for the last chunk's drain.
"""

import math
import os
import sys

import numpy as np

sys.path.insert(0, "/opt/trn_rl_repo")

import ml_dtypes  # noqa: E402

BF16 = ml_dtypes.bfloat16

B, CIN, COUT, H, W, KS, NH = 8, 128, 128, 64, 64, 3, 4
A = CIN // NH          # 32
OSH = COUT // NH       # 32
K2 = KS * KS           # 9
NPIX = H * W           # 4096
PW = W + 2             # 66 padded width
PH = H + 2
NPAD = PW * PH + PW + 2  # slack so shifted strided views stay in-bounds
NCHUNK = 8
CHUNK = NPIX // NCHUNK  # 512
ROWS_PER_CHUNK = H // NCHUNK  # 8
SCALE = A ** (-0.5)
PCK = NPIX // CIN      # 32 packed-stat columns per row

_CACHE = {}


def _pos_encoding_np():
    pos = np.arange(K2, dtype=np.float32)[:, None]
    div = np.exp(np.arange(0, CIN, 2, dtype=np.float32) * (-math.log(10000.0) / CIN))
    ang = pos * div[None, :]
    return np.stack([np.sin(ang), np.cos(ang)], -1).reshape(K2, CIN)


def _host_fold(ln_g, ln_b, Wq, bq, Wk, bk, Wv, bv, Wp, bp, Wf, bf):
    """All weight-space precomputation (f64 for accuracy, cast at the end)."""
    g = ln_g.astype(np.float64)
    b = ln_b.astype(np.float64)
    Wq = Wq.astype(np.float64); Wk = Wk.astype(np.float64)
    Wv = Wv.astype(np.float64); Wp = Wp.astype(np.float64)
    Wf = Wf.astype(np.float64)
    bq = bq.astype(np.float64); bk = bk.astype(np.float64)
    bv = bv.astype(np.float64); bp = bp.astype(np.float64)
    bfv = bf.astype(np.float64)

    Wq_ = g[:, None] * Wq; bq_ = b @ Wq + bq
    Wk_ = g[:, None] * Wk; bk_ = b @ Wk + bk
    Wv_ = g[:, None] * Wv; bv_ = b @ Wv + bv

    pos = _pos_encoding_np().astype(np.float64) @ Wp + bp  # [K2, NH*A]
    pos = pos.reshape(K2, NH, A)

    # pos-scores: row layout (n,k) = n*9+k ; scores_pos = z @ Wqs + bqs, scaled
    Wqs = np.zeros((CIN, NH * K2))
    bqs = np.zeros((NH * K2,))
    Wq_r = Wq_.reshape(CIN, NH, A)
    bq_r = bq_.reshape(NH, A)
    for n in range(NH):
        for k in range(K2):
            Wqs[:, n * K2 + k] = Wq_r[:, n, :] @ pos[k, n, :]
            bqs[n * K2 + k] = bq_r[n, :] @ pos[k, n, :]
    Wqs *= SCALE
    bqs *= SCALE

    # BD_k [CIN, 36]: (n,a) x (n*9+k) = SCALE ; concat over k -> [128, 9*36]
    bd = np.zeros((K2, CIN, NH * K2))
    for k in range(K2):
        for n in range(NH):
            bd[k, n * A:(n + 1) * A, n * K2 + k] = SCALE
    bd = np.concatenate([bd[k] for k in range(K2)], axis=1)  # [128, 324]

    # E_k [36, 128]: (n*9+k') x (n,o) = 1 iff k'==k ; concat -> [36, 9*128]
    ek = np.zeros((K2, NH * K2, CIN))
    for k in range(K2):
        for n in range(NH):
            ek[k, n * K2 + k, n * OSH:(n + 1) * OSH] = 1.0
    ek = np.concatenate([ek[k] for k in range(K2)], axis=1)  # [36, 1152]

    # RS36 [36, 36]: (n*9+k) x (n'*9+k') = 1 iff n==n'  (sum over k, rep over k')
    rs = np.zeros((NH * K2, NH * K2))
    for n in range(NH):
        rs[n * K2:(n + 1) * K2, n * K2:(n + 1) * K2] = 1.0

    def pad128(m):
        out = np.zeros((CIN, m.shape[1]))
        out[:m.shape[0]] = m
        return out

    # one concatenated bf16 const blob [128, 2061]:
    # wq(0:128) wk(128:256) wv(256:384) wqs(384:420) bd(420:744) wf(744:872)
    # ones(872:873) ek(873:2025) rs(2025:2061)
    cb16 = np.concatenate([
        Wq_, Wk_, Wv_, Wqs, bd, Wf, np.ones((CIN, 1)), pad128(ek), pad128(rs),
        np.eye(CIN), np.ones((CIN, CIN)),
    ], axis=1).astype(BF16)
    # f32 bias blob [128, 5]: bqc bkc bvc bfb bqsc(pad)
    bfv2 = bfv + (1.0 + K2 * 1e-8) * (bv_ @ Wf)
    cf32 = np.stack([
        bq_, bk_, bv_, bfv2, np.concatenate([bqs, np.zeros(CIN - NH * K2)]),
        np.ones(CIN),
    ], axis=1).astype(np.float32)
    return {"cb16": np.ascontiguousarray(cb16), "cf32": np.ascontiguousarray(cf32)}


def _shift_delta(k):
    di, dj = k // KS - 1, k % KS - 1
    return di * PW + dj


def _build_bass():
    import concourse.bass as bass
    import concourse.tile as tile
    from concourse import bacc, mybir

    f32 = mybir.dt.float32
    bf16 = mybir.dt.bfloat16
    AF = mybir.ActivationFunctionType

    nc = bacc.Bacc("TRN2", target_bir_lowering=False, debug=False)

    ext = {}
    ext["x"] = nc.dram_tensor("x", [CIN, NPIX], f32, kind="ExternalInput")
    ext["cb16"] = nc.dram_tensor("cb16", [CIN, 2317], bf16, kind="ExternalInput")
    ext["cf32"] = nc.dram_tensor("cf32", [CIN, 6], f32, kind="ExternalInput")
    out_ext = nc.dram_tensor("out", [COUT, NPIX], f32, kind="ExternalOutput")

    with tile.TileContext(nc) as tc:
        _kernel_body(tc, nc, mybir, f32, bf16, AF, bass, ext, out_ext)

    nc.compile()
    return nc


def _kernel_body(tc, nc, mybir, f32, bf16, AF, bass, ext, out_ext):
    from contextlib import ExitStack

    f32r = mybir.dt.float32r
    mult = mybir.AluOpType.mult
    sub = mybir.AluOpType.subtract

    ctx = ExitStack()
    with ctx:
        consts = ctx.enter_context(tc.tile_pool(name="consts", bufs=1))
        big = ctx.enter_context(tc.tile_pool(name="big", bufs=1))
        xbfp = ctx.enter_context(tc.tile_pool(name="xbf", bufs=2))
        sqp = ctx.enter_context(tc.tile_pool(name="sqp", bufs=2))
        tmpp = ctx.enter_context(tc.tile_pool(name="tmpp", bufs=5))
        zp = ctx.enter_context(tc.tile_pool(name="zp", bufs=6))
        qp_pool = ctx.enter_context(tc.tile_pool(name="qpool", bufs=6))
        pkp = ctx.enter_context(tc.tile_pool(name="pkp", bufs=8))
        mkp = ctx.enter_context(tc.tile_pool(name="mkp", bufs=8))
        repp = ctx.enter_context(tc.tile_pool(name="repp", bufs=8))
        smallp = ctx.enter_context(tc.tile_pool(name="small", bufs=5))
        statp = ctx.enter_context(tc.tile_pool(name="statp", bufs=1))
        dramp = ctx.enter_context(tc.tile_pool(name="drams", bufs=1, space="DRAM"))
        outp = ctx.enter_context(tc.tile_pool(name="outp", bufs=6))
        ps_a = ctx.enter_context(tc.tile_pool(name="ps_a", bufs=1, space="PSUM"))
        ps_s = ctx.enter_context(tc.tile_pool(name="ps_s", bufs=2, space="PSUM"))
        ps_r = ctx.enter_context(tc.tile_pool(name="ps_r", bufs=3, space="PSUM"))
        ps_o = ctx.enter_context(tc.tile_pool(name="ps_o", bufs=2, space="PSUM"))

        def mm(out, lhsT, rhs, **kw):
            nc.tensor.matmul(out, lhsT, rhs, **kw)

        # ---- big SBUF buffers ----
        x_sb = big.tile([CIN, NPIX], f32)
        k_pad = big.tile([CIN, NPAD], bf16)
        v_pad = big.tile([CIN, NPAD], bf16)
        smb = big.tile([CIN, 2 * NPIX], bf16)  # [rstd | mu] broadcast cols

        # ---- preload ACT tables with dummy ops on a zeroed scratch ----
        scr = statp.tile([1, 4], f32, tag="scr")
        nc.vector.memset(scr[:], 1.0)
        nc.scalar.square(scr[:, 1:2], scr[:, 0:1])
        nc.scalar.sqrt(scr[:, 2:3], scr[:, 0:1])
        nc.scalar.activation(scr[:, 3:4], scr[:, 0:1], AF.Exp)
        nc.scalar.copy(scr[:, 1:2], scr[:, 0:1])
        nc.scalar.add(scr[:, 2:3], scr[:, 0:1], scr[:, 0:1])

        # ---- input + constants (x quarter 0 first, then consts) ----
        nc.scalar.dma_start(out=x_sb[:, 0:CHUNK], in_=ext["x"][:, 0:CHUNK])
        nc.scalar.dma_start(out=x_sb[:, CHUNK:NPIX // 4],
                            in_=ext["x"][:, CHUNK:NPIX // 4])
        cb16 = consts.tile([CIN, 2317], bf16)
        nc.sync.dma_start(cb16[:], ext["cb16"][:])
        cf32 = consts.tile([CIN, 6], f32)
        nc.sync.dma_start(cf32[:], ext["cf32"][:])
        for qx in range(1, 4):
            sl = slice(qx * NPIX // 4, (qx + 1) * NPIX // 4)
            nc.scalar.dma_start(out=x_sb[:, sl], in_=ext["x"][:, sl])
        wq = cb16[:, 0:128]
        wk = cb16[:, 128:256]
        wv = cb16[:, 256:384]
        wqs = cb16[:, 384:420]
        bdw = cb16[:, 420:744]
        wf = cb16[:, 744:872]
        ones_k = cb16[:, 872:873]
        ekw = cb16[0:NH * K2, 873:2025]
        rsw = cb16[0:NH * K2, 2025:2061]
        ident = cb16[:, 2061:2189]
        ones_row = cb16[0:1, 2189:2317]
        ident16 = cb16[0:1, 2061:2062]
        bqc = cf32[:, 0:1]
        bkc = cf32[:, 1:2]
        bvc = cf32[:, 2:3]
        bfb = cf32[:, 3:4]
        bqsc = cf32[0:NH * K2, 4:5]
        one32 = cf32[0:1, 5:6]
        ones32r = cf32[:, 5:6].bitcast(mybir.dt.float32r)

        # ---- stats, issued per quarter so LN finalize overlaps later chunks ----
        # s12row: single row, s1 at [0, j], s2 at [0, NPIX + j]
        s12row = statp.tile([1, 2 * NPIX], f32, tag="s12row")
        s_dram = dramp.tile([2, NPIX], bf16)
        QPIX = NPIX // 4      # 1024 pixels per quarter
        QCK = QPIX // CIN     # 8 packed columns per quarter

        def stats_chunk(c):
            sl = slice(c * CHUNK, (c + 1) * CHUNK)
            x_bf = xbfp.tile([CIN, CHUNK], bf16, tag="xbf")
            nc.gpsimd.tensor_copy(x_bf[:], x_sb[:, sl])            # Pool
            yield
            sq_bf = sqp.tile([CIN, CHUNK], bf16, tag="sq")
            nc.scalar.square(sq_bf[:], x_bf[:])                    # ACT
            yield
            s1 = ps_s.tile([1, CHUNK], f32, tag="pss")
            mm(s1[:], ones_k, x_bf[:], start=True, stop=True)
            yield
            s2 = ps_s.tile([1, CHUNK], f32, tag="pss")
            mm(s2[:], ones_k, sq_bf[:], start=True, stop=True)
            yield
            nc.vector.tensor_copy(s12row[0:1, sl], s1[:])          # DVE evict
            yield
            s2dst = s12row[0:1, NPIX + c * CHUNK:NPIX + (c + 1) * CHUNK]
            if c % 2 == 0:
                nc.scalar.copy(s2dst, s2[:])                       # ACT evict
            else:
                nc.vector.tensor_copy(s2dst, s2[:])                # DVE evict
            yield

        def stats_quarter(qr):
            yield from stats_chunk(2 * qr)
            yield from stats_chunk(2 * qr + 1)
            yield from stats_finalize(qr)

        def fin_pe(g):
            # PE-path LN finalize for head chunk g (pixels g*512..g*512+511):
            # pack via transposes, math, transpose rows back, bcast matmuls
            # into PSUM (rbps/mbps) read directly by the centering ops.
            GC = 4  # 512 px / 128
            base = g * CHUNK
            tps = ps_o.tile([CIN, 2 * GC], f32, tag="acc")
            for j in range(GC):
                o1 = base + j * CIN
                nc.tensor.transpose(tps[:, j:j + 1],
                                    s12row[0:1, o1:o1 + CIN], one32)
                o2 = NPIX + base + j * CIN
                nc.tensor.transpose(tps[:, GC + j:GC + j + 1],
                                    s12row[0:1, o2:o2 + CIN], one32)
            yield
            S1 = tps[:, 0:GC]
            S2 = tps[:, GC:2 * GC]
            stat2 = statp.tile([CIN, 3 * GC], f32, tag=f"fpe{g}")
            mean = stat2[:, 0:GC]
            msq = stat2[:, GC:2 * GC]
            var = stat2[:, 2 * GC:3 * GC]
            nc.vector.tensor_scalar_mul(mean[:], S1[:], 1.0 / CIN)
            yield
            nc.vector.tensor_tensor(msq[:], mean[:], mean[:], mult)
            nc.vector.scalar_tensor_tensor(var[:], S2[:], 1.0 / CIN, msq[:],
                                           mult, sub)
            nc.vector.tensor_scalar_add(var[:], var[:], 1e-5)
            yield
            stdg = statp.tile([CIN, GC], f32, tag=f"fpestd{g}")
            nc.scalar.sqrt(stdg[:], var[:])
            yield
            rstdg = statp.tile([CIN, GC], f32, tag=f"fper{g}")
            nc.vector.reciprocal_approx_fast(rstdg[:], stdg[:])
            yield
            sbfg = statp.tile([CIN, 2 * GC], bf16, tag=f"fpeb{g}")
            nc.vector.tensor_copy(sbfg[:, 0:GC], rstdg[:])
            nc.vector.tensor_copy(sbfg[:, GC:2 * GC], mean[:])
            yield
            # rows: T[j,p]: j 0-3 rstd segments, 4-7 mean segments
            tr = ps_o.tile([2 * GC, CIN], bf16, tag="acc")
            nc.tensor.transpose(tr[:], sbfg[:], ident)
            yield
            srow8 = statp.tile([1, 2 * GC * CIN], bf16, tag=f"fpes{g}")
            engs = [nc.scalar, nc.vector, nc.gpsimd]
            for j in range(2 * GC):
                eng = engs[j % 3]
                if eng is nc.vector:
                    eng.tensor_copy(srow8[0:1, j * CIN:(j + 1) * CIN],
                                    tr[j:j + 1, :])
                elif eng is nc.gpsimd:
                    eng.tensor_copy(srow8[0:1, j * CIN:(j + 1) * CIN],
                                    tr[j:j + 1, :])
                else:
                    eng.copy(srow8[0:1, j * CIN:(j + 1) * CIN], tr[j:j + 1, :])
            yield
            rbp = ps_r.tile([CIN, CHUNK], f32, tag="rep")
            mbp = ps_r.tile([CIN, CHUNK], f32, tag="rep")
            rbps[g], mbps[g] = rbp, mbp
            for j in range(GC):
                mm(rbp[:, j * CIN:(j + 1) * CIN], ones_row,
                   srow8[0:1, j * CIN:(j + 1) * CIN], start=True, stop=True)
                mm(mbp[:, j * CIN:(j + 1) * CIN], ones_row,
                   srow8[0:1, (GC + j) * CIN:(GC + j + 1) * CIN],
                   start=True, stop=True)
            yield

        def transpose_pack(qr, tps, half):
            for j in range(half * QCK // 2, (half + 1) * QCK // 2):
                o1 = qr * QPIX + j * CIN
                nc.tensor.transpose(tps[:, j:j + 1],
                                    s12row[0:1, o1:o1 + CIN], one32)
                o2 = NPIX + qr * QPIX + j * CIN
                nc.tensor.transpose(tps[:, QCK + j:QCK + j + 1],
                                    s12row[0:1, o2:o2 + CIN], one32)

        def stats_finalize(qr):
            qsl = slice(qr * QPIX, (qr + 1) * QPIX)
            qsl2 = slice(NPIX + qr * QPIX, NPIX + (qr + 1) * QPIX)
            # pack quarter via PE transposes: tps[p, b*QCK+j] = s_b[qr*1024+j*128+p]
            tps = ps_s.tile([CIN, 2 * QCK], f32, tag="pss")
            transpose_pack(qr, tps, 0)
            yield
            transpose_pack(qr, tps, 1)
            yield
            S1 = tps[:, 0:QCK]
            S2 = tps[:, QCK:2 * QCK]
            stat2 = statp.tile([CIN, 3 * QCK], f32, tag=f"stat2{qr}")
            mean = stat2[:, 0:QCK]
            msq = stat2[:, QCK:2 * QCK]
            var = stat2[:, 2 * QCK:3 * QCK]
            nc.vector.tensor_scalar_mul(mean[:], S1[:], 1.0 / CIN)
            yield
            nc.vector.tensor_tensor(msq[:], mean[:], mean[:], mult)
            nc.vector.scalar_tensor_tensor(var[:], S2[:], 1.0 / CIN, msq[:], mult, sub)
            nc.vector.tensor_scalar_add(var[:], var[:], 1e-5)
            yield
            std = statp.tile([CIN, QCK], f32, tag=f"std{qr}")
            nc.scalar.sqrt(std[:], var[:])
            rstd32 = statp.tile([CIN, QCK], f32, tag=f"rstd32{qr}")
            nc.vector.reciprocal_approx_fast(rstd32[:], std[:])
            stat_bf = statp.tile([CIN, 2 * QCK], bf16, tag=f"stat_bf{qr}")
            nc.vector.tensor_copy(stat_bf[:, 0:QCK], rstd32[:])
            yield
            nc.vector.tensor_copy(stat_bf[:, QCK:2 * QCK], mean[:])
            yield
            # DMAs to DRAM rows; pixel index = qr*1024 + j*128 + p
            dd0 = s_dram[0:1, 0:1]
            for row, scols in ((0, slice(0, QCK)), (1, slice(QCK, 2 * QCK))):
                ddst = bass.AP(tensor=dd0.tensor,
                               offset=dd0.offset + row * NPIX + qr * QPIX,
                               ap=[[1, CIN], [CIN, QCK]])
                nc.sync.dma_start(ddst, stat_bf[:, scols])
                yield
            # partition-broadcast back into smb ([rstd | mu] column blocks)
            dd = s_dram[0:1, 0:1]
            for row, dcols in ((0, qsl), (1, qsl2)):
                src = bass.AP(tensor=dd.tensor,
                              offset=dd.offset + row * NPIX + qr * QPIX,
                              ap=[[0, CIN], [1, QPIX]])
                nc.sync.dma_start(smb[:, dcols], src)
                yield

        for pad_t in (k_pad, v_pad):
            nc.gpsimd.memset(pad_t[:, 0:PW + 1], 0.0)
            nc.gpsimd.memset(
                pad_t[:, PW + 65:PW + 65 + 64 * PW].rearrange(
                    "p (r t) -> p r t", t=PW)[:, :, 0:2], 0.0)
            nc.gpsimd.memset(pad_t[:, 65 * PW + 1:NPAD], 0.0)

        z_tiles = [None] * NCHUNK
        q_tiles = [None] * NCHUNK

        def pad_view(t, c, delta=0):
            off = (1 + c * ROWS_PER_CHUNK) * PW + 1 + delta
            return t[:, off:off + ROWS_PER_CHUNK * PW].rearrange(
                "p (r w) -> p r w", r=ROWS_PER_CHUNK, w=PW)[:, :, 0:W]

        def proj_gen(c):
            sl = slice(c * CHUNK, (c + 1) * CHUNK)
            tmp = tmpp.tile([CIN, CHUNK], bf16, tag="tmp")
            nc.vector.tensor_tensor(
                tmp[:], x_sb[:, sl],
                smb[:, NPIX + c * CHUNK:NPIX + (c + 1) * CHUNK], sub)
            yield
            z = zp.tile([CIN, CHUNK], bf16, tag="z")
            z_tiles[c] = z
            nc.vector.tensor_tensor(z[:], tmp[:], smb[:, sl], mult)
            yield
            qps = ps_a.tile([CIN, CHUNK], f32, tag="ps_a")
            mm(qps[:], wq, z[:], start=True, stop=True)
            yield
            q_c = qp_pool.tile([CIN, CHUNK], bf16, tag="q")
            q_tiles[c] = q_c
            nc.scalar.add(q_c[:], qps[:], bqc)                  # ACT
            yield
            kps = ps_a.tile([CIN, CHUNK], f32, tag="ps_a")
            mm(kps[:], wk, z[:], start=True, stop=True)
            yield
            nc.scalar.add(pad_view(k_pad, c)[:],
                          kps[:].rearrange("p (r w) -> p r w",
                                           r=ROWS_PER_CHUNK, w=W), bkc)  # ACT
            yield
            vps = ps_a.tile([CIN, CHUNK], f32, tag="ps_a")
            mm(vps[:], wv, z[:], start=True, stop=True)
            yield
            nc.scalar.copy(pad_view(v_pad, c)[:],
                           vps[:].rearrange("p (r w) -> p r w",
                                            r=ROWS_PER_CHUNK, w=W))  # ACT
            yield

        def scores_gen(c):
            q_v = q_tiles[c][:].rearrange("p (r w) -> p r w", r=ROWS_PER_CHUNK, w=W)
            sc = ps_s.tile([NH * K2, CHUNK], f32, tag="pss")
            mm(sc[:], wqs, z_tiles[c][:], start=True, stop=False)
            yield
            for k in range(K2):
                pk = pkp.tile([CIN, CHUNK], bf16, tag="pk")
                pk_v = pk[:].rearrange("p (r w) -> p r w", r=ROWS_PER_CHUNK, w=W)
                eng = nc.gpsimd if k >= 5 else nc.vector
                eng.tensor_tensor(pk_v[:], q_v[:],
                                  pad_view(k_pad, c, _shift_delta(k))[:], mult)
                yield
                mm(sc[:], cb16[:, 420 + k * NH * K2:420 + (k + 1) * NH * K2],
                   pk[:], start=False, stop=(k == K2 - 1))
                yield
            exp_c = smallp.tile([NH * K2, CHUNK], bf16, tag="exp")
            nc.scalar.activation(exp_c[:], sc[:], AF.Exp, bias=bqsc)  # ACT
            yield
            dn = ps_s.tile([NH * K2, CHUNK], f32, tag="pss")
            mm(dn[:], rsw, exp_c[:], start=True, stop=True)
            yield
            rcp = smallp.tile([NH * K2, CHUNK], f32, tag="rcp")
            nc.vector.reciprocal_approx_fast(rcp[:], dn[:])
            yield
            rcp_bf = smallp.tile([NH * K2, CHUNK], bf16, tag="rcpbf")
            nc.scalar.copy(rcp_bf[:], rcp[:])                       # ACT
            yield
            attn_c = smallp.tile([NH * K2, CHUNK], bf16, tag="attn")
            nc.vector.tensor_tensor(attn_c[:], exp_c[:], rcp_bf[:], mult)
            attn_tiles[c] = attn_c
            yield

        def av_gen(c, split=False):
            sl = slice(c * CHUNK, (c + 1) * CHUNK)
            attn_c = attn_tiles[c]
            acc = ps_o.tile([COUT, CHUNK], f32, tag="acc")
            acc2 = None
            if split:
                acc2 = ps_a.tile([COUT, CHUNK], f32, tag="ps_a")
            # rep matmuls write bf16 PSUM (exact: 0/1 matrix x bf16 attn);
            # mk multiplies read PSUM directly as 2-byte packed operands.
            # 'e': ACT-evict + DVE bf16 mult for a couple of ks to offload DVE.
            modes = ['e', 'd', 'e', 'd', 'e', 'd', 'e', 'd', 'e']
            for k in range(K2):
                rep = ps_r.tile([CIN, CHUNK], f32, tag="rep")
                mm(rep[:], cb16[0:NH * K2, 873 + k * CIN:873 + (k + 1) * CIN],
                   attn_c[:], start=True, stop=True)
                yield
                mk = mkp.tile([CIN, CHUNK], bf16, tag="mk")
                mk_v = mk[:].rearrange("p (r w) -> p r w", r=ROWS_PER_CHUNK, w=W)
                vv = pad_view(v_pad, c, _shift_delta(k))
                if modes[k] == 'd':
                    nc.vector.tensor_tensor(
                        mk_v[:], rep[:].rearrange("p (r w) -> p r w",
                                                  r=ROWS_PER_CHUNK, w=W),
                        vv[:], mult)
                    yield
                else:
                    rep_sb = repp.tile([CIN, CHUNK], bf16, tag="repsb")
                    nc.scalar.copy(rep_sb[:], rep[:])
                    yield
                    nc.vector.tensor_tensor(
                        mk_v[:], rep_sb[:].rearrange("p (r w) -> p r w",
                                                     r=ROWS_PER_CHUNK, w=W),
                        vv[:], mult)
                    yield
                if split and k % 2 == 1:
                    mm(acc2[:], wf, mk[:], start=(k == 1), stop=(k == K2 - 2))
                else:
                    mm(acc[:], wf, mk[:], start=(k == 0), stop=(k == K2 - 1))
                yield
            out_sb = outp.tile([COUT, CHUNK], f32, tag="outsb")
            nc.scalar.add(out_sb[:], acc[:], bfb)                # ACT
            if split:
                yield
                nc.vector.tensor_tensor(out_sb[:], out_sb[:], acc2[:],
                                        mybir.AluOpType.add)
            yield
            nc.sync.dma_start(out_ext[:, sl], out_sb[:])
            yield

        def av_half(c, h):
            # 256-px half-chunk AV chain for the drain: all mults read rep
            # PSUM directly on DVE; two halves interleave to halve the tail.
            HP = CHUNK // 2
            hsl = slice(c * CHUNK + h * HP, c * CHUNK + (h + 1) * HP)
            attn_c = attn_tiles[c]
            if h == 0:
                acc = ps_o.tile([COUT, CHUNK // 2], f32, tag="acc")
            else:
                acc = ps_a.tile([COUT, CHUNK // 2], f32, tag="ps_a")
            for k in range(K2):
                rep = ps_r.tile([CIN, HP], f32, tag="rep")
                mm(rep[:], cb16[0:NH * K2, 873 + k * CIN:873 + (k + 1) * CIN],
                   attn_c[:, h * HP:(h + 1) * HP], start=True, stop=True)
                yield
                mk = mkp.tile([CIN, HP], bf16, tag="mk")
                off = (1 + c * ROWS_PER_CHUNK + h * 4) * PW + 1 + _shift_delta(k)
                vv = v_pad[:, off:off + 4 * PW].rearrange(
                    "p (r w) -> p r w", r=4, w=PW)[:, :, 0:W]
                nc.vector.tensor_tensor(
                    mk[:].rearrange("p (r w) -> p r w", r=4, w=W),
                    rep[:].rearrange("p (r w) -> p r w", r=4, w=W),
                    vv[:], mult)
                yield
                mm(acc[:], wf, mk[:], start=(k == 0), stop=(k == K2 - 1))
                yield
            out_sb = outp.tile([COUT, HP], f32, tag=f"outh{h}")
            nc.scalar.add(out_sb[:], acc[:], bfb)                # ACT
            yield
            nc.sync.dma_start(out_ext[:, hsl], out_sb[:])
            yield

        def run_all(gens):
            gens = [g for g in gens if g is not None]
            while gens:
                alive = []
                for g in gens:
                    try:
                        next(g)
                        alive.append(g)
                    except StopIteration:
                        pass
                gens = alive

        attn_tiles = [None] * NCHUNK
        for qr in range(3):
            run_all([stats_quarter(qr)])
        run_all([stats_quarter(3), proj_gen(0), proj_gen(1)])
        run_all([scores_gen(0), proj_gen(2)])
        # steady 2-deep software pipeline: SCORES(c) | AV(c-1) | PROJ(c+2)
        for c in range(1, NCHUNK):
            run_all([scores_gen(c), av_gen(c - 1),
                     proj_gen(c + 2) if c + 2 < NCHUNK else None])
        run_all([av_gen(NCHUNK - 1, split=True)])


def _get_compiled():
    if "nc" not in _CACHE:
        _CACHE["nc"] = _build_bass()
    return _CACHE["nc"]


def kernel(**inputs):
    x = np.asarray(inputs["x"], dtype=np.float32)          # [B, CIN, H, W]
    consts = _host_fold(
        np.asarray(inputs["ln_g"]), np.asarray(inputs["ln_b"]),
        np.asarray(inputs["Wq"]), np.asarray(inputs["bq"]),
        np.asarray(inputs["Wk"]), np.asarray(inputs["bk"]),
        np.asarray(inputs["Wv"]), np.asarray(inputs["bv"]),
        np.asarray(inputs["Wp"]), np.asarray(inputs["bp"]),
        np.asarray(inputs["Wf"]), np.asarray(inputs["bf"]),
    )

    nc = _get_compiled()

    from concourse.bass_utils import run_bass_kernel_spmd

    core_ids = list(range(B))
    in_maps = []
    for i in range(B):
        m = {"x": np.ascontiguousarray(x[i].reshape(CIN, NPIX))}
        m.update(consts)
        in_maps.append(m)

    res = run_bass_kernel_spmd(nc, in_maps, core_ids,
                               trace=bool(int(os.environ.get("KTRACE", "0"))))
    _CACHE["last_result"] = res
    out = np.stack([res.results[i]["out"].reshape(COUT, H, W) for i in range(B)])
    return out.astype(np.float32)


if __name__ == "__main__":
    nc = _get_compiled()
    print("compiled OK")


# revision 20
# speedup vs baseline: 1.0084x; 1.0084x over previous
"""AttentionConv2D (3x3 windowed multi-head attention) on 8 TRN2 NeuronCores. v2.

Sharding: data-parallel over batch (B=8 -> 1 image per core), weights replicated.
Per-core layout: channel-major [128 ch, 4096 pix].

v2 vs v1: no aug matmuls (pre-centered z via DMA partition-broadcast of mu/rstd
rows), biases folded into ACT evictions / Exp bias, SBUF-SBUF stats
redistribution, engine-balanced elementwise work, software-pipelined chunks.
"""

import math
import os
import sys

import numpy as np

sys.path.insert(0, "/opt/trn_rl_repo")

import ml_dtypes  # noqa: E402

BF16 = ml_dtypes.bfloat16

B, CIN, COUT, H, W, KS, NH = 8, 128, 128, 64, 64, 3, 4
A = CIN // NH          # 32
OSH = COUT // NH       # 32
K2 = KS * KS           # 9
NPIX = H * W           # 4096
PW = W + 2             # 66 padded width
PH = H + 2
NPAD = PW * PH + PW + 2  # slack so shifted strided views stay in-bounds
NCHUNK = 8
CHUNK = NPIX // NCHUNK  # 512
ROWS_PER_CHUNK = H // NCHUNK  # 8
SCALE = A ** (-0.5)
PCK = NPIX // CIN      # 32 packed-stat columns per row

_CACHE = {}


def _pos_encoding_np():
    pos = np.arange(K2, dtype=np.float32)[:, None]
    div = np.exp(np.arange(0, CIN, 2, dtype=np.float32) * (-math.log(10000.0) / CIN))
    ang = pos * div[None, :]
    return np.stack([np.sin(ang), np.cos(ang)], -1).reshape(K2, CIN)


def _host_fold(ln_g, ln_b, Wq, bq, Wk, bk, Wv, bv, Wp, bp, Wf, bf):
    """All weight-space precomputation (f64 for accuracy, cast at the end)."""
    g = ln_g.astype(np.float64)
    b = ln_b.astype(np.float64)
    Wq = Wq.astype(np.float64); Wk = Wk.astype(np.float64)
    Wv = Wv.astype(np.float64); Wp = Wp.astype(np.float64)
    Wf = Wf.astype(np.float64)
    bq = bq.astype(np.float64); bk = bk.astype(np.float64)
    bv = bv.astype(np.float64); bp = bp.astype(np.float64)
    bfv = bf.astype(np.float64)

    Wq_ = g[:, None] * Wq; bq_ = b @ Wq + bq
    Wk_ = g[:, None] * Wk; bk_ = b @ Wk + bk
    Wv_ = g[:, None] * Wv; bv_ = b @ Wv + bv

    pos = _pos_encoding_np().astype(np.float64) @ Wp + bp  # [K2, NH*A]
    pos = pos.reshape(K2, NH, A)

    # pos-scores: row layout (n,k) = n*9+k ; scores_pos = z @ Wqs + bqs, scaled
    Wqs = np.zeros((CIN, NH * K2))
    bqs = np.zeros((NH * K2,))
    Wq_r = Wq_.reshape(CIN, NH, A)
    bq_r = bq_.reshape(NH, A)
    for n in range(NH):
        for k in range(K2):
            Wqs[:, n * K2 + k] = Wq_r[:, n, :] @ pos[k, n, :]
            bqs[n * K2 + k] = bq_r[n, :] @ pos[k, n, :]
    Wqs *= SCALE
    bqs *= SCALE

    # BD_k [CIN, 36]: (n,a) x (n*9+k) = SCALE ; concat over k -> [128, 9*36]
    bd = np.zeros((K2, CIN, NH * K2))
    for k in range(K2):
        for n in range(NH):
            bd[k, n * A:(n + 1) * A, n * K2 + k] = SCALE
    bd = np.concatenate([bd[k] for k in range(K2)], axis=1)  # [128, 324]

    # E_k [36, 128]: (n*9+k') x (n,o) = 1 iff k'==k ; concat -> [36, 9*128]
    ek = np.zeros((K2, NH * K2, CIN))
    for k in range(K2):
        for n in range(NH):
            ek[k, n * K2 + k, n * OSH:(n + 1) * OSH] = 1.0
    ek = np.concatenate([ek[k] for k in range(K2)], axis=1)  # [36, 1152]

    # RS36 [36, 36]: (n*9+k) x (n'*9+k') = 1 iff n==n'  (sum over k, rep over k')
    rs = np.zeros((NH * K2, NH * K2))
    for n in range(NH):
        rs[n * K2:(n + 1) * K2, n * K2:(n + 1) * K2] = 1.0

    def pad128(m):
        out = np.zeros((CIN, m.shape[1]))
        out[:m.shape[0]] = m
        return out

    # one concatenated bf16 const blob [128, 2061]:
    # wq(0:128) wk(128:256) wv(256:384) wqs(384:420) bd(420:744) wf(744:872)
    # ones(872:873) ek(873:2025) rs(2025:2061)
    cb16 = np.concatenate([
        Wq_, Wk_, Wv_, Wqs, bd, Wf, np.ones((CIN, 1)), pad128(ek), pad128(rs),
        np.eye(CIN), np.ones((CIN, CIN)),
    ], axis=1).astype(BF16)
    # f32 bias blob [128, 5]: bqc bkc bvc bfb bqsc(pad)
    bfv2 = bfv + (1.0 + K2 * 1e-8) * (bv_ @ Wf)
    cf32 = np.stack([
        bq_, bk_, bv_, bfv2, np.concatenate([bqs, np.zeros(CIN - NH * K2)]),
        np.ones(CIN),
    ], axis=1).astype(np.float32)
    return {"cb16": np.ascontiguousarray(cb16), "cf32": np.ascontiguousarray(cf32)}


def _shift_delta(k):
    di, dj = k // KS - 1, k % KS - 1
    return di * PW + dj


def _build_bass():
    import concourse.bass as bass
    import concourse.tile as tile
    from concourse import bacc, mybir

    f32 = mybir.dt.float32
    bf16 = mybir.dt.bfloat16
    AF = mybir.ActivationFunctionType

    nc = bacc.Bacc("TRN2", target_bir_lowering=False, debug=False)

    ext = {}
    ext["x"] = nc.dram_tensor("x", [CIN, NPIX], f32, kind="ExternalInput")
    ext["cb16"] = nc.dram_tensor("cb16", [CIN, 2317], bf16, kind="ExternalInput")
    ext["cf32"] = nc.dram_tensor("cf32", [CIN, 6], f32, kind="ExternalInput")
    out_ext = nc.dram_tensor("out", [COUT, NPIX], f32, kind="ExternalOutput")

    with tile.TileContext(nc) as tc:
        _kernel_body(tc, nc, mybir, f32, bf16, AF, bass, ext, out_ext)

    nc.compile()
    return nc


def _kernel_body(tc, nc, mybir, f32, bf16, AF, bass, ext, out_ext):
    from contextlib import ExitStack

    f32r = mybir.dt.float32r
    mult = mybir.AluOpType.mult
    sub = mybir.AluOpType.subtract

    ctx = ExitStack()
    with ctx:
        consts = ctx.enter_context(tc.tile_pool(name="consts", bufs=1))
        big = ctx.enter_context(tc.tile_pool(name="big", bufs=1))
        xbfp = ctx.enter_context(tc.tile_pool(name="xbf", bufs=2))
        sqp = ctx.enter_context(tc.tile_pool(name="sqp", bufs=2))
        tmpp = ctx.enter_context(tc.tile_pool(name="tmpp", bufs=5))
        zp = ctx.enter_context(tc.tile_pool(name="zp", bufs=6))
        qp_pool = ctx.enter_context(tc.tile_pool(name="qpool", bufs=6))
        pkp = ctx.enter_context(tc.tile_pool(name="pkp", bufs=8))
        mkp = ctx.enter_context(tc.tile_pool(name="mkp", bufs=8))
        repp = ctx.enter_context(tc.tile_pool(name="repp", bufs=8))
        smallp = ctx.enter_context(tc.tile_pool(name="small", bufs=5))
        statp = ctx.enter_context(tc.tile_pool(name="statp", bufs=1))
        dramp = ctx.enter_context(tc.tile_pool(name="drams", bufs=1, space="DRAM"))
        outp = ctx.enter_context(tc.tile_pool(name="outp", bufs=4))
        ps_a = ctx.enter_context(tc.tile_pool(name="ps_a", bufs=1, space="PSUM"))
        ps_s = ctx.enter_context(tc.tile_pool(name="ps_s", bufs=2, space="PSUM"))
        ps_r = ctx.enter_context(tc.tile_pool(name="ps_r", bufs=3, space="PSUM"))
        ps_o = ctx.enter_context(tc.tile_pool(name="ps_o", bufs=2, space="PSUM"))

        def mm(out, lhsT, rhs, **kw):
            nc.tensor.matmul(out, lhsT, rhs, **kw)

        # ---- big SBUF buffers ----
        x_sb = big.tile([CIN, NPIX], f32)
        k_pad = big.tile([CIN, NPAD], bf16)
        v_pad = big.tile([CIN, NPAD], bf16)
        smb = big.tile([CIN, 2 * NPIX], bf16)  # [rstd | mu] broadcast cols

        # ---- preload ACT tables with dummy ops on a zeroed scratch ----
        scr = statp.tile([1, 4], f32, tag="scr")
        nc.vector.memset(scr[:], 1.0)
        nc.scalar.square(scr[:, 1:2], scr[:, 0:1])
        nc.scalar.sqrt(scr[:, 2:3], scr[:, 0:1])
        nc.scalar.activation(scr[:, 3:4], scr[:, 0:1], AF.Exp)
        nc.scalar.copy(scr[:, 1:2], scr[:, 0:1])
        nc.scalar.add(scr[:, 2:3], scr[:, 0:1], scr[:, 0:1])

        # ---- input + constants (x quarter 0 first, then consts) ----
        nc.scalar.dma_start(out=x_sb[:, 0:CHUNK], in_=ext["x"][:, 0:CHUNK])
        nc.scalar.dma_start(out=x_sb[:, CHUNK:NPIX // 4],
                            in_=ext["x"][:, CHUNK:NPIX // 4])
        cb16 = consts.tile([CIN, 2317], bf16)
        nc.sync.dma_start(cb16[:], ext["cb16"][:])
        cf32 = consts.tile([CIN, 6], f32)
        nc.sync.dma_start(cf32[:], ext["cf32"][:])
        for qx in range(1, 4):
            sl = slice(qx * NPIX // 4, (qx + 1) * NPIX // 4)
            nc.scalar.dma_start(out=x_sb[:, sl], in_=ext["x"][:, sl])
        wq = cb16[:, 0:128]
        wk = cb16[:, 128:256]
        wv = cb16[:, 256:384]
        wqs = cb16[:, 384:420]
        bdw = cb16[:, 420:744]
        wf = cb16[:, 744:872]
        ones_k = cb16[:, 872:873]
        ekw = cb16[0:NH * K2, 873:2025]
        rsw = cb16[0:NH * K2, 2025:2061]
        ident = cb16[:, 2061:2189]
        ones_row = cb16[0:1, 2189:2317]
        ident16 = cb16[0:1, 2061:2062]
        bqc = cf32[:, 0:1]
        bkc = cf32[:, 1:2]
        bvc = cf32[:, 2:3]
        bfb = cf32[:, 3:4]
        bqsc = cf32[0:NH * K2, 4:5]
        one32 = cf32[0:1, 5:6]
        ones32r = cf32[:, 5:6].bitcast(mybir.dt.float32r)

        # ---- stats, issued per quarter so LN finalize overlaps later chunks ----
        # s12row: single row, s1 at [0, j], s2 at [0, NPIX + j]
        s12row = statp.tile([1, 2 * NPIX], f32, tag="s12row")
        s_dram = dramp.tile([2, NPIX], bf16)
        QPIX = NPIX // 4      # 1024 pixels per quarter
        QCK = QPIX // CIN     # 8 packed columns per quarter

        def stats_chunk(c):
            sl = slice(c * CHUNK, (c + 1) * CHUNK)
            x_bf = xbfp.tile([CIN, CHUNK], bf16, tag="xbf")
            nc.gpsimd.tensor_copy(x_bf[:], x_sb[:, sl])            # Pool
            yield
            sq_bf = sqp.tile([CIN, CHUNK], bf16, tag="sq")
            nc.scalar.square(sq_bf[:], x_bf[:])                    # ACT
            yield
            s1 = ps_s.tile([1, CHUNK], f32, tag="pss")
            mm(s1[:], ones_k, x_bf[:], start=True, stop=True)
            yield
            s2 = ps_s.tile([1, CHUNK], f32, tag="pss")
            mm(s2[:], ones_k, sq_bf[:], start=True, stop=True)
            yield
            nc.vector.tensor_copy(s12row[0:1, sl], s1[:])          # DVE evict
            yield
            s2dst = s12row[0:1, NPIX + c * CHUNK:NPIX + (c + 1) * CHUNK]
            if c % 2 == 0:
                nc.scalar.copy(s2dst, s2[:])                       # ACT evict
            else:
                nc.vector.tensor_copy(s2dst, s2[:])                # DVE evict
            yield

        def stats_quarter(qr):
            yield from stats_chunk(2 * qr)
            yield from stats_chunk(2 * qr + 1)
            yield from stats_finalize(qr)

        def fin_pe(g):
            # PE-path LN finalize for head chunk g (pixels g*512..g*512+511):
            # pack via transposes, math, transpose rows back, bcast matmuls
            # into PSUM (rbps/mbps) read directly by the centering ops.
            GC = 4  # 512 px / 128
            base = g * CHUNK
            tps = ps_o.tile([CIN, 2 * GC], f32, tag="acc")
            for j in range(GC):
                o1 = base + j * CIN
                nc.tensor.transpose(tps[:, j:j + 1],
                                    s12row[0:1, o1:o1 + CIN], one32)
                o2 = NPIX + base + j * CIN
                nc.tensor.transpose(tps[:, GC + j:GC + j + 1],
                                    s12row[0:1, o2:o2 + CIN], one32)
            yield
            S1 = tps[:, 0:GC]
            S2 = tps[:, GC:2 * GC]
            stat2 = statp.tile([CIN, 3 * GC], f32, tag=f"fpe{g}")
            mean = stat2[:, 0:GC]
            msq = stat2[:, GC:2 * GC]
            var = stat2[:, 2 * GC:3 * GC]
            nc.vector.tensor_scalar_mul(mean[:], S1[:], 1.0 / CIN)
            yield
            nc.vector.tensor_tensor(msq[:], mean[:], mean[:], mult)
            nc.vector.scalar_tensor_tensor(var[:], S2[:], 1.0 / CIN, msq[:],
                                           mult, sub)
            nc.vector.tensor_scalar_add(var[:], var[:], 1e-5)
            yield
            stdg = statp.tile([CIN, GC], f32, tag=f"fpestd{g}")
            nc.scalar.sqrt(stdg[:], var[:])
            yield
            rstdg = statp.tile([CIN, GC], f32, tag=f"fper{g}")
            nc.vector.reciprocal_approx_fast(rstdg[:], stdg[:])
            yield
            sbfg = statp.tile([CIN, 2 * GC], bf16, tag=f"fpeb{g}")
            nc.vector.tensor_copy(sbfg[:, 0:GC], rstdg[:])
            nc.vector.tensor_copy(sbfg[:, GC:2 * GC], mean[:])
            yield
            # rows: T[j,p]: j 0-3 rstd segments, 4-7 mean segments
            tr = ps_o.tile([2 * GC, CIN], bf16, tag="acc")
            nc.tensor.transpose(tr[:], sbfg[:], ident)
            yield
            srow8 = statp.tile([1, 2 * GC * CIN], bf16, tag=f"fpes{g}")
            engs = [nc.scalar, nc.vector, nc.gpsimd]
            for j in range(2 * GC):
                eng = engs[j % 3]
                if eng is nc.vector:
                    eng.tensor_copy(srow8[0:1, j * CIN:(j + 1) * CIN],
                                    tr[j:j + 1, :])
                elif eng is nc.gpsimd:
                    eng.tensor_copy(srow8[0:1, j * CIN:(j + 1) * CIN],
                                    tr[j:j + 1, :])
                else:
                    eng.copy(srow8[0:1, j * CIN:(j + 1) * CIN], tr[j:j + 1, :])
            yield
            rbp = ps_r.tile([CIN, CHUNK], f32, tag="rep")
            mbp = ps_r.tile([CIN, CHUNK], f32, tag="rep")
            rbps[g], mbps[g] = rbp, mbp
            for j in range(GC):
                mm(rbp[:, j * CIN:(j + 1) * CIN], ones_row,
                   srow8[0:1, j * CIN:(j + 1) * CIN], start=True, stop=True)
                mm(mbp[:, j * CIN:(j + 1) * CIN], ones_row,
                   srow8[0:1, (GC + j) * CIN:(GC + j + 1) * CIN],
                   start=True, stop=True)
            yield

        def transpose_pack(qr, tps, half):
            for j in range(half * QCK // 2, (half + 1) * QCK // 2):
                o1 = qr * QPIX + j * CIN
                nc.tensor.transpose(tps[:, j:j + 1],
                                    s12row[0:1, o1:o1 + CIN], one32)
                o2 = NPIX + qr * QPIX + j * CIN
                nc.tensor.transpose(tps[:, QCK + j:QCK + j + 1],
                                    s12row[0:1, o2:o2 + CIN], one32)

        def stats_finalize(qr):
            qsl = slice(qr * QPIX, (qr + 1) * QPIX)
            qsl2 = slice(NPIX + qr * QPIX, NPIX + (qr + 1) * QPIX)
            # pack quarter via PE transposes: tps[p, b*QCK+j] = s_b[qr*1024+j*128+p]
            tps = ps_s.tile([CIN, 2 * QCK], f32, tag="pss")
            transpose_pack(qr, tps, 0)
            yield
            transpose_pack(qr, tps, 1)
            yield
            S1 = tps[:, 0:QCK]
            S2 = tps[:, QCK:2 * QCK]
            stat2 = statp.tile([CIN, 3 * QCK], f32, tag=f"stat2{qr}")
            mean = stat2[:, 0:QCK]
            msq = stat2[:, QCK:2 * QCK]
            var = stat2[:, 2 * QCK:3 * QCK]
            nc.vector.tensor_scalar_mul(mean[:], S1[:], 1.0 / CIN)
            yield
            nc.vector.tensor_tensor(msq[:], mean[:], mean[:], mult)
            nc.vector.scalar_tensor_tensor(var[:], S2[:], 1.0 / CIN, msq[:], mult, sub)
            nc.vector.tensor_scalar_add(var[:], var[:], 1e-5)
            yield
            std = statp.tile([CIN, QCK], f32, tag=f"std{qr}")
            nc.scalar.sqrt(std[:], var[:])
            rstd32 = statp.tile([CIN, QCK], f32, tag=f"rstd32{qr}")
            nc.vector.reciprocal_approx_fast(rstd32[:], std[:])
            stat_bf = statp.tile([CIN, 2 * QCK], bf16, tag=f"stat_bf{qr}")
            nc.vector.tensor_copy(stat_bf[:, 0:QCK], rstd32[:])
            yield
            nc.vector.tensor_copy(stat_bf[:, QCK:2 * QCK], mean[:])
            yield
            # DMAs to DRAM rows; pixel index = qr*1024 + j*128 + p
            dd0 = s_dram[0:1, 0:1]
            for row, scols in ((0, slice(0, QCK)), (1, slice(QCK, 2 * QCK))):
                ddst = bass.AP(tensor=dd0.tensor,
                               offset=dd0.offset + row * NPIX + qr * QPIX,
                               ap=[[1, CIN], [CIN, QCK]])
                nc.sync.dma_start(ddst, stat_bf[:, scols])
                yield
            # partition-broadcast back into smb ([rstd | mu] column blocks)
            dd = s_dram[0:1, 0:1]
            for row, dcols in ((0, qsl), (1, qsl2)):
                src = bass.AP(tensor=dd.tensor,
                              offset=dd.offset + row * NPIX + qr * QPIX,
                              ap=[[0, CIN], [1, QPIX]])
                nc.sync.dma_start(smb[:, dcols], src)
                yield

        for pad_t in (k_pad, v_pad):
            nc.gpsimd.memset(pad_t[:, 0:PW + 1], 0.0)
            nc.gpsimd.memset(
                pad_t[:, PW + 65:PW + 65 + 64 * PW].rearrange(
                    "p (r t) -> p r t", t=PW)[:, :, 0:2], 0.0)
            nc.gpsimd.memset(pad_t[:, 65 * PW + 1:NPAD], 0.0)

        z_tiles = [None] * NCHUNK
        q_tiles = [None] * NCHUNK

        def pad_view(t, c, delta=0):
            off = (1 + c * ROWS_PER_CHUNK) * PW + 1 + delta
            return t[:, off:off + ROWS_PER_CHUNK * PW].rearrange(
                "p (r w) -> p r w", r=ROWS_PER_CHUNK, w=PW)[:, :, 0:W]

        def proj_gen(c):
            sl = slice(c * CHUNK, (c + 1) * CHUNK)
            tmp = tmpp.tile([CIN, CHUNK], bf16, tag="tmp")
            nc.vector.tensor_tensor(
                tmp[:], x_sb[:, sl],
                smb[:, NPIX + c * CHUNK:NPIX + (c + 1) * CHUNK], sub)
            yield
            z = zp.tile([CIN, CHUNK], bf16, tag="z")
            z_tiles[c] = z
            nc.vector.tensor_tensor(z[:], tmp[:], smb[:, sl], mult)
            yield
            qps = ps_a.tile([CIN, CHUNK], f32, tag="ps_a")
            mm(qps[:], wq, z[:], start=True, stop=True)
            yield
            q_c = qp_pool.tile([CIN, CHUNK], bf16, tag="q")
            q_tiles[c] = q_c
            nc.scalar.add(q_c[:], qps[:], bqc)                  # ACT
            yield
            kps = ps_a.tile([CIN, CHUNK], f32, tag="ps_a")
            mm(kps[:], wk, z[:], start=True, stop=True)
            yield
            nc.scalar.add(pad_view(k_pad, c)[:],
                          kps[:].rearrange("p (r w) -> p r w",
                                           r=ROWS_PER_CHUNK, w=W), bkc)  # ACT
            yield
            vps = ps_a.tile([CIN, CHUNK], f32, tag="ps_a")
            mm(vps[:], wv, z[:], start=True, stop=True)
            yield
            nc.scalar.copy(pad_view(v_pad, c)[:],
                           vps[:].rearrange("p (r w) -> p r w",
                                            r=ROWS_PER_CHUNK, w=W))  # ACT
            yield

        def scores_gen(c):
            q_v = q_tiles[c][:].rearrange("p (r w) -> p r w", r=ROWS_PER_CHUNK, w=W)
            sc = ps_s.tile([NH * K2, CHUNK], f32, tag="pss")
            mm(sc[:], wqs, z_tiles[c][:], start=True, stop=False)
            yield
            for k in range(K2):
                pk = pkp.tile([CIN, CHUNK], bf16, tag="pk")
                pk_v = pk[:].rearrange("p (r w) -> p r w", r=ROWS_PER_CHUNK, w=W)
                eng = nc.gpsimd if k >= 5 else nc.vector
                eng.tensor_tensor(pk_v[:], q_v[:],
                                  pad_view(k_pad, c, _shift_delta(k))[:], mult)
                yield
                mm(sc[:], cb16[:, 420 + k * NH * K2:420 + (k + 1) * NH * K2],
                   pk[:], start=False, stop=(k == K2 - 1))
                yield
            exp_c = smallp.tile([NH * K2, CHUNK], bf16, tag="exp")
            nc.scalar.activation(exp_c[:], sc[:], AF.Exp, bias=bqsc)  # ACT
            yield
            dn = ps_s.tile([NH * K2, CHUNK], f32, tag="pss")
            mm(dn[:], rsw, exp_c[:], start=True, stop=True)
            yield
            rcp = smallp.tile([NH * K2, CHUNK], f32, tag="rcp")
            nc.vector.reciprocal_approx_fast(rcp[:], dn[:])
            yield
            rcp_bf = smallp.tile([NH * K2, CHUNK], bf16, tag="rcpbf")
            nc.scalar.copy(rcp_bf[:], rcp[:])                       # ACT
            yield
            attn_c = smallp.tile([NH * K2, CHUNK], bf16, tag="attn")
            nc.vector.tensor_tensor(attn_c[:], exp_c[:], rcp_bf[:], mult)
            attn_tiles[c] = attn_c
            yield

        def av_gen(c, split=False):
            sl = slice(c * CHUNK, (c + 1) * CHUNK)
            attn_c = attn_tiles[c]
            if split:
                acc = ps_o.tile([COUT, CHUNK // 2], f32, tag="acc")
                accR = ps_a.tile([COUT, CHUNK // 2], f32, tag="ps_a")
            else:
                acc = ps_o.tile([COUT, CHUNK], f32, tag="acc")
            # rep matmuls write bf16 PSUM (exact: 0/1 matrix x bf16 attn);
            # mk multiplies read PSUM directly as 2-byte packed operands.
            # 'e': ACT-evict + DVE bf16 mult for a couple of ks to offload DVE.
            modes = ['e', 'd', 'e', 'd', 'e', 'd', 'e', 'd', 'e']
            for k in range(K2):
                rep = ps_r.tile([CIN, CHUNK], f32, tag="rep")
                mm(rep[:], cb16[0:NH * K2, 873 + k * CIN:873 + (k + 1) * CIN],
                   attn_c[:], start=True, stop=True)
                yield
                mk = mkp.tile([CIN, CHUNK], bf16, tag="mk")
                mk_v = mk[:].rearrange("p (r w) -> p r w", r=ROWS_PER_CHUNK, w=W)
                vv = pad_view(v_pad, c, _shift_delta(k))
                if modes[k] == 'd':
                    nc.vector.tensor_tensor(
                        mk_v[:], rep[:].rearrange("p (r w) -> p r w",
                                                  r=ROWS_PER_CHUNK, w=W),
                        vv[:], mult)
                    yield
                else:
                    rep_sb = repp.tile([CIN, CHUNK], bf16, tag="repsb")
                    nc.scalar.copy(rep_sb[:], rep[:])
                    yield
                    nc.vector.tensor_tensor(
                        mk_v[:], rep_sb[:].rearrange("p (r w) -> p r w",
                                                     r=ROWS_PER_CHUNK, w=W),
                        vv[:], mult)
                    yield
                if split:
                    HC = CHUNK // 2
                    mm(acc[:], wf, mk[:, 0:HC],
                       start=(k == 0), stop=(k == K2 - 1))
                    yield
                    mm(accR[:], wf, mk[:, HC:CHUNK],
                       start=(k == 0), stop=(k == K2 - 1))
                else:
                    mm(acc[:], wf, mk[:], start=(k == 0), stop=(k == K2 - 1))
                yield
            out_sb = outp.tile([COUT, CHUNK], f32, tag="outsb")
            if split:
                HC = CHUNK // 2
                nc.scalar.add(out_sb[:, 0:HC], acc[:], bfb)         # ACT
                yield
                nc.sync.dma_start(out_ext[:, c * CHUNK:c * CHUNK + HC],
                                  out_sb[:, 0:HC])
                yield
                nc.scalar.add(out_sb[:, HC:CHUNK], accR[:], bfb)
                yield
                nc.sync.dma_start(out_ext[:, c * CHUNK + HC:(c + 1) * CHUNK],
                                  out_sb[:, HC:CHUNK])
            else:
                nc.scalar.add(out_sb[:], acc[:], bfb)            # ACT
                yield
                nc.sync.dma_start(out_ext[:, sl], out_sb[:])
            yield

        def av_half(c, h):
            # 256-px half-chunk AV chain for the drain: all mults read rep
            # PSUM directly on DVE; two halves interleave to halve the tail.
            HP = CHUNK // 2
            hsl = slice(c * CHUNK + h * HP, c * CHUNK + (h + 1) * HP)
            attn_c = attn_tiles[c]
            if h == 0:
                acc = ps_o.tile([COUT, CHUNK // 2], f32, tag="acc")
            else:
                acc = ps_a.tile([COUT, CHUNK // 2], f32, tag="ps_a")
            for k in range(K2):
                rep = ps_r.tile([CIN, HP], f32, tag="rep")
                mm(rep[:], cb16[0:NH * K2, 873 + k * CIN:873 + (k + 1) * CIN],
                   attn_c[:, h * HP:(h + 1) * HP], start=True, stop=True)
                yield
                mk = mkp.tile([CIN, HP], bf16, tag="mk")
                off = (1 + c * ROWS_PER_CHUNK + h * 4) * PW + 1 + _shift_delta(k)
                vv = v_pad[:, off:off + 4 * PW].rearrange(
                    "p (r w) -> p r w", r=4, w=PW)[:, :, 0:W]
                nc.vector.tensor_tensor(
                    mk[:].rearrange("p (r w) -> p r w", r=4, w=W),
                    rep[:].rearrange("p (r w) -> p r w", r=4, w=W),
                    vv[:], mult)
                yield
                mm(acc[:], wf, mk[:], start=(k == 0), stop=(k == K2 - 1))
                yield
            out_sb = outp.tile([COUT, HP], f32, tag=f"outh{h}")
            nc.scalar.add(out_sb[:], acc[:], bfb)                # ACT
            yield
            nc.sync.dma_start(out_ext[:, hsl], out_sb[:])
            yield

        def run_all(gens):
            gens = [g for g in gens if g is not None]
            while gens:
                alive = []
                for g in gens:
                    try:
                        next(g)
                        alive.append(g)
                    except StopIteration:
                        pass
                gens = alive

        attn_tiles = [None] * NCHUNK
        for qr in range(3):
            run_all([stats_quarter(qr)])
        run_all([stats_quarter(3), proj_gen(0), proj_gen(1)])
        run_all([scores_gen(0), proj_gen(2)])
        # steady 2-deep software pipeline: SCORES(c) | AV(c-1) | PROJ(c+2)
        for c in range(1, NCHUNK):
            run_all([scores_gen(c), av_gen(c - 1),
                     proj_gen(c + 2) if c + 2 < NCHUNK else None])
        run_all([av_gen(NCHUNK - 1, split=True)])


def _get_compiled():
    if "nc" not in _CACHE:
        _CACHE["nc"] = _build_bass()
    return _CACHE["nc"]


def kernel(**inputs):
    x = np.asarray(inputs["x"], dtype=np.float32)          # [B, CIN, H, W]
    consts = _host_fold(
        np.asarray(inputs["ln_g"]), np.asarray(inputs["ln_b"]),
        np.asarray(inputs["Wq"]), np.asarray(inputs["bq"]),
        np.asarray(inputs["Wk"]), np.asarray(inputs["bk"]),
        np.asarray(inputs["Wv"]), np.asarray(inputs["bv"]),
        np.asarray(inputs["Wp"]), np.asarray(inputs["bp"]),
        np.asarray(inputs["Wf"]), np.asarray(inputs["bf"]),
    )

    nc = _get_compiled()

    from concourse.bass_utils import run_bass_kernel_spmd

    core_ids = list(range(B))
    in_maps = []
    for i in range(B):
        m = {"x": np.ascontiguousarray(x[i].reshape(CIN, NPIX))}
        m.update(consts)
        in_maps.append(m)

    res = run_bass_kernel_spmd(nc, in_maps, core_ids,
                               trace=bool(int(os.environ.get("KTRACE", "0"))))
    _CACHE["last_result"] = res
    out = np.stack([res.results[i]["out"].reshape(COUT, H, W) for i in range(B)])
    return out.astype(np.float32)


if __name__ == "__main__":
    nc = _get_compiled()
    print("compiled OK")


# revision 21
# speedup vs baseline: 1.0089x; 1.0006x over previous
"""AttentionConv2D (3x3 windowed multi-head attention) on 8 TRN2 NeuronCores. v2.

Sharding: data-parallel over batch (B=8 -> 1 image per core), weights replicated.
Per-core layout: channel-major [128 ch, 4096 pix].

v2 vs v1: no aug matmuls (pre-centered z via DMA partition-broadcast of mu/rstd
rows), biases folded into ACT evictions / Exp bias, SBUF-SBUF stats
redistribution, engine-balanced elementwise work, software-pipelined chunks.
"""

import math
import os
import sys

import numpy as np

sys.path.insert(0, "/opt/trn_rl_repo")

import ml_dtypes  # noqa: E402

BF16 = ml_dtypes.bfloat16

B, CIN, COUT, H, W, KS, NH = 8, 128, 128, 64, 64, 3, 4
A = CIN // NH          # 32
OSH = COUT // NH       # 32
K2 = KS * KS           # 9
NPIX = H * W           # 4096
PW = W + 2             # 66 padded width
PH = H + 2
NPAD = PW * PH + PW + 2  # slack so shifted strided views stay in-bounds
NCHUNK = 8
CHUNK = NPIX // NCHUNK  # 512
ROWS_PER_CHUNK = H // NCHUNK  # 8
SCALE = A ** (-0.5)
PCK = NPIX // CIN      # 32 packed-stat columns per row

_CACHE = {}


def _pos_encoding_np():
    pos = np.arange(K2, dtype=np.float32)[:, None]
    div = np.exp(np.arange(0, CIN, 2, dtype=np.float32) * (-math.log(10000.0) / CIN))
    ang = pos * div[None, :]
    return np.stack([np.sin(ang), np.cos(ang)], -1).reshape(K2, CIN)


def _host_fold(ln_g, ln_b, Wq, bq, Wk, bk, Wv, bv, Wp, bp, Wf, bf):
    """All weight-space precomputation (f64 for accuracy, cast at the end)."""
    g = ln_g.astype(np.float64)
    b = ln_b.astype(np.float64)
    Wq = Wq.astype(np.float64); Wk = Wk.astype(np.float64)
    Wv = Wv.astype(np.float64); Wp = Wp.astype(np.float64)
    Wf = Wf.astype(np.float64)
    bq = bq.astype(np.float64); bk = bk.astype(np.float64)
    bv = bv.astype(np.float64); bp = bp.astype(np.float64)
    bfv = bf.astype(np.float64)

    Wq_ = g[:, None] * Wq; bq_ = b @ Wq + bq
    Wk_ = g[:, None] * Wk; bk_ = b @ Wk + bk
    Wv_ = g[:, None] * Wv; bv_ = b @ Wv + bv

    pos = _pos_encoding_np().astype(np.float64) @ Wp + bp  # [K2, NH*A]
    pos = pos.reshape(K2, NH, A)

    # pos-scores: row layout (n,k) = n*9+k ; scores_pos = z @ Wqs + bqs, scaled
    Wqs = np.zeros((CIN, NH * K2))
    bqs = np.zeros((NH * K2,))
    Wq_r = Wq_.reshape(CIN, NH, A)
    bq_r = bq_.reshape(NH, A)
    for n in range(NH):
        for k in range(K2):
            Wqs[:, n * K2 + k] = Wq_r[:, n, :] @ pos[k, n, :]
            bqs[n * K2 + k] = bq_r[n, :] @ pos[k, n, :]
    Wqs *= SCALE
    bqs *= SCALE

    # BD_k [CIN, 36]: (n,a) x (n*9+k) = SCALE ; concat over k -> [128, 9*36]
    bd = np.zeros((K2, CIN, NH * K2))
    for k in range(K2):
        for n in range(NH):
            bd[k, n * A:(n + 1) * A, n * K2 + k] = SCALE
    bd = np.concatenate([bd[k] for k in range(K2)], axis=1)  # [128, 324]

    # E_k [36, 128]: (n*9+k') x (n,o) = 1 iff k'==k ; concat -> [36, 9*128]
    ek = np.zeros((K2, NH * K2, CIN))
    for k in range(K2):
        for n in range(NH):
            ek[k, n * K2 + k, n * OSH:(n + 1) * OSH] = 1.0
    ek = np.concatenate([ek[k] for k in range(K2)], axis=1)  # [36, 1152]

    # RS36 [36, 36]: (n*9+k) x (n'*9+k') = 1 iff n==n'  (sum over k, rep over k')
    rs = np.zeros((NH * K2, NH * K2))
    for n in range(NH):
        rs[n * K2:(n + 1) * K2, n * K2:(n + 1) * K2] = 1.0

    def pad128(m):
        out = np.zeros((CIN, m.shape[1]))
        out[:m.shape[0]] = m
        return out

    # one concatenated bf16 const blob [128, 2061]:
    # wq(0:128) wk(128:256) wv(256:384) wqs(384:420) bd(420:744) wf(744:872)
    # ones(872:873) ek(873:2025) rs(2025:2061)
    cb16 = np.concatenate([
        Wq_, Wk_, Wv_, Wqs, bd, Wf, np.ones((CIN, 1)), pad128(ek), pad128(rs),
        np.eye(CIN), np.ones((CIN, CIN)),
    ], axis=1).astype(BF16)
    # f32 bias blob [128, 5]: bqc bkc bvc bfb bqsc(pad)
    bfv2 = bfv + (1.0 + K2 * 1e-8) * (bv_ @ Wf)
    cf32 = np.stack([
        bq_, bk_, bv_, bfv2, np.concatenate([bqs, np.zeros(CIN - NH * K2)]),
        np.ones(CIN),
    ], axis=1).astype(np.float32)
    return {"cb16": np.ascontiguousarray(cb16), "cf32": np.ascontiguousarray(cf32)}


def _shift_delta(k):
    di, dj = k // KS - 1, k % KS - 1
    return di * PW + dj


def _build_bass():
    import concourse.bass as bass
    import concourse.tile as tile
    from concourse import bacc, mybir

    f32 = mybir.dt.float32
    bf16 = mybir.dt.bfloat16
    AF = mybir.ActivationFunctionType

    nc = bacc.Bacc("TRN2", target_bir_lowering=False, debug=False)

    ext = {}
    ext["x"] = nc.dram_tensor("x", [CIN, NPIX], f32, kind="ExternalInput")
    ext["cb16"] = nc.dram_tensor("cb16", [CIN, 2317], bf16, kind="ExternalInput")
    ext["cf32"] = nc.dram_tensor("cf32", [CIN, 6], f32, kind="ExternalInput")
    out_ext = nc.dram_tensor("out", [COUT, NPIX], f32, kind="ExternalOutput")

    with tile.TileContext(nc) as tc:
        _kernel_body(tc, nc, mybir, f32, bf16, AF, bass, ext, out_ext)

    nc.compile()
    return nc


def _kernel_body(tc, nc, mybir, f32, bf16, AF, bass, ext, out_ext):
    from contextlib import ExitStack

    f32r = mybir.dt.float32r
    mult = mybir.AluOpType.mult
    sub = mybir.AluOpType.subtract

    ctx = ExitStack()
    with ctx:
        consts = ctx.enter_context(tc.tile_pool(name="consts", bufs=1))
        big = ctx.enter_context(tc.tile_pool(name="big", bufs=1))
        xbfp = ctx.enter_context(tc.tile_pool(name="xbf", bufs=2))
        sqp = ctx.enter_context(tc.tile_pool(name="sqp", bufs=2))
        tmpp = ctx.enter_context(tc.tile_pool(name="tmpp", bufs=5))
        zp = ctx.enter_context(tc.tile_pool(name="zp", bufs=6))
        qp_pool = ctx.enter_context(tc.tile_pool(name="qpool", bufs=6))
        pkp = ctx.enter_context(tc.tile_pool(name="pkp", bufs=8))
        mkp = ctx.enter_context(tc.tile_pool(name="mkp", bufs=8))
        repp = ctx.enter_context(tc.tile_pool(name="repp", bufs=8))
        smallp = ctx.enter_context(tc.tile_pool(name="small", bufs=5))
        statp = ctx.enter_context(tc.tile_pool(name="statp", bufs=1))
        dramp = ctx.enter_context(tc.tile_pool(name="drams", bufs=1, space="DRAM"))
        outp = ctx.enter_context(tc.tile_pool(name="outp", bufs=4))
        ps_a = ctx.enter_context(tc.tile_pool(name="ps_a", bufs=1, space="PSUM"))
        ps_s = ctx.enter_context(tc.tile_pool(name="ps_s", bufs=2, space="PSUM"))
        ps_r = ctx.enter_context(tc.tile_pool(name="ps_r", bufs=3, space="PSUM"))
        ps_o = ctx.enter_context(tc.tile_pool(name="ps_o", bufs=2, space="PSUM"))

        def mm(out, lhsT, rhs, **kw):
            nc.tensor.matmul(out, lhsT, rhs, **kw)

        # ---- big SBUF buffers ----
        x_sb = big.tile([CIN, NPIX], f32)
        k_pad = big.tile([CIN, NPAD], bf16)
        v_pad = big.tile([CIN, NPAD], bf16)
        smb = big.tile([CIN, 2 * NPIX], bf16)  # [rstd | mu] broadcast cols

        # ---- preload ACT tables with dummy ops on a zeroed scratch ----
        scr = statp.tile([1, 4], f32, tag="scr")
        nc.vector.memset(scr[:], 1.0)
        nc.scalar.square(scr[:, 1:2], scr[:, 0:1])
        nc.scalar.sqrt(scr[:, 2:3], scr[:, 0:1])
        nc.scalar.activation(scr[:, 3:4], scr[:, 0:1], AF.Exp)
        nc.scalar.copy(scr[:, 1:2], scr[:, 0:1])
        nc.scalar.add(scr[:, 2:3], scr[:, 0:1], scr[:, 0:1])

        # ---- input + constants (x quarter 0 first, then consts) ----
        nc.sync.dma_start(x_sb[:, 0:CHUNK], ext["x"][:, 0:CHUNK])
        nc.sync.dma_start(x_sb[:, CHUNK:NPIX // 4], ext["x"][:, CHUNK:NPIX // 4])
        cb16 = consts.tile([CIN, 2317], bf16)
        nc.scalar.dma_start(out=cb16[:], in_=ext["cb16"][:])
        cf32 = consts.tile([CIN, 6], f32)
        nc.scalar.dma_start(out=cf32[:], in_=ext["cf32"][:])
        for qx in range(1, 4):
            sl = slice(qx * NPIX // 4, (qx + 1) * NPIX // 4)
            nc.scalar.dma_start(out=x_sb[:, sl], in_=ext["x"][:, sl])
        wq = cb16[:, 0:128]
        wk = cb16[:, 128:256]
        wv = cb16[:, 256:384]
        wqs = cb16[:, 384:420]
        bdw = cb16[:, 420:744]
        wf = cb16[:, 744:872]
        ones_k = cb16[:, 872:873]
        ekw = cb16[0:NH * K2, 873:2025]
        rsw = cb16[0:NH * K2, 2025:2061]
        ident = cb16[:, 2061:2189]
        ones_row = cb16[0:1, 2189:2317]
        ident16 = cb16[0:1, 2061:2062]
        bqc = cf32[:, 0:1]
        bkc = cf32[:, 1:2]
        bvc = cf32[:, 2:3]
        bfb = cf32[:, 3:4]
        bqsc = cf32[0:NH * K2, 4:5]
        one32 = cf32[0:1, 5:6]
        ones32r = cf32[:, 5:6].bitcast(mybir.dt.float32r)

        # ---- stats, issued per quarter so LN finalize overlaps later chunks ----
        # s12row: single row, s1 at [0, j], s2 at [0, NPIX + j]
        s12row = statp.tile([1, 2 * NPIX], f32, tag="s12row")
        s_dram = dramp.tile([2, NPIX], bf16)
        QPIX = NPIX // 4      # 1024 pixels per quarter
        QCK = QPIX // CIN     # 8 packed columns per quarter

        def stats_chunk(c):
            sl = slice(c * CHUNK, (c + 1) * CHUNK)
            x_bf = xbfp.tile([CIN, CHUNK], bf16, tag="xbf")
            nc.gpsimd.tensor_copy(x_bf[:], x_sb[:, sl])            # Pool
            yield
            sq_bf = sqp.tile([CIN, CHUNK], bf16, tag="sq")
            nc.scalar.square(sq_bf[:], x_bf[:])                    # ACT
            yield
            s1 = ps_s.tile([1, CHUNK], f32, tag="pss")
            mm(s1[:], ones_k, x_bf[:], start=True, stop=True)
            yield
            s2 = ps_s.tile([1, CHUNK], f32, tag="pss")
            mm(s2[:], ones_k, sq_bf[:], start=True, stop=True)
            yield
            nc.vector.tensor_copy(s12row[0:1, sl], s1[:])          # DVE evict
            yield
            s2dst = s12row[0:1, NPIX + c * CHUNK:NPIX + (c + 1) * CHUNK]
            if c % 2 == 0:
                nc.scalar.copy(s2dst, s2[:])                       # ACT evict
            else:
                nc.vector.tensor_copy(s2dst, s2[:])                # DVE evict
            yield

        def stats_quarter(qr):
            yield from stats_chunk(2 * qr)
            yield from stats_chunk(2 * qr + 1)
            yield from stats_finalize(qr)

        def fin_pe(g):
            # PE-path LN finalize for head chunk g (pixels g*512..g*512+511):
            # pack via transposes, math, transpose rows back, bcast matmuls
            # into PSUM (rbps/mbps) read directly by the centering ops.
            GC = 4  # 512 px / 128
            base = g * CHUNK
            tps = ps_o.tile([CIN, 2 * GC], f32, tag="acc")
            for j in range(GC):
                o1 = base + j * CIN
                nc.tensor.transpose(tps[:, j:j + 1],
                                    s12row[0:1, o1:o1 + CIN], one32)
                o2 = NPIX + base + j * CIN
                nc.tensor.transpose(tps[:, GC + j:GC + j + 1],
                                    s12row[0:1, o2:o2 + CIN], one32)
            yield
            S1 = tps[:, 0:GC]
            S2 = tps[:, GC:2 * GC]
            stat2 = statp.tile([CIN, 3 * GC], f32, tag=f"fpe{g}")
            mean = stat2[:, 0:GC]
            msq = stat2[:, GC:2 * GC]
            var = stat2[:, 2 * GC:3 * GC]
            nc.vector.tensor_scalar_mul(mean[:], S1[:], 1.0 / CIN)
            yield
            nc.vector.tensor_tensor(msq[:], mean[:], mean[:], mult)
            nc.vector.scalar_tensor_tensor(var[:], S2[:], 1.0 / CIN, msq[:],
                                           mult, sub)
            nc.vector.tensor_scalar_add(var[:], var[:], 1e-5)
            yield
            stdg = statp.tile([CIN, GC], f32, tag=f"fpestd{g}")
            nc.scalar.sqrt(stdg[:], var[:])
            yield
            rstdg = statp.tile([CIN, GC], f32, tag=f"fper{g}")
            nc.vector.reciprocal_approx_fast(rstdg[:], stdg[:])
            yield
            sbfg = statp.tile([CIN, 2 * GC], bf16, tag=f"fpeb{g}")
            nc.vector.tensor_copy(sbfg[:, 0:GC], rstdg[:])
            nc.vector.tensor_copy(sbfg[:, GC:2 * GC], mean[:])
            yield
            # rows: T[j,p]: j 0-3 rstd segments, 4-7 mean segments
            tr = ps_o.tile([2 * GC, CIN], bf16, tag="acc")
            nc.tensor.transpose(tr[:], sbfg[:], ident)
            yield
            srow8 = statp.tile([1, 2 * GC * CIN], bf16, tag=f"fpes{g}")
            engs = [nc.scalar, nc.vector, nc.gpsimd]
            for j in range(2 * GC):
                eng = engs[j % 3]
                if eng is nc.vector:
                    eng.tensor_copy(srow8[0:1, j * CIN:(j + 1) * CIN],
                                    tr[j:j + 1, :])
                elif eng is nc.gpsimd:
                    eng.tensor_copy(srow8[0:1, j * CIN:(j + 1) * CIN],
                                    tr[j:j + 1, :])
                else:
                    eng.copy(srow8[0:1, j * CIN:(j + 1) * CIN], tr[j:j + 1, :])
            yield
            rbp = ps_r.tile([CIN, CHUNK], f32, tag="rep")
            mbp = ps_r.tile([CIN, CHUNK], f32, tag="rep")
            rbps[g], mbps[g] = rbp, mbp
            for j in range(GC):
                mm(rbp[:, j * CIN:(j + 1) * CIN], ones_row,
                   srow8[0:1, j * CIN:(j + 1) * CIN], start=True, stop=True)
                mm(mbp[:, j * CIN:(j + 1) * CIN], ones_row,
                   srow8[0:1, (GC + j) * CIN:(GC + j + 1) * CIN],
                   start=True, stop=True)
            yield

        def transpose_pack(qr, tps, half):
            for j in range(half * QCK // 2, (half + 1) * QCK // 2):
                o1 = qr * QPIX + j * CIN
                nc.tensor.transpose(tps[:, j:j + 1],
                                    s12row[0:1, o1:o1 + CIN], one32)
                o2 = NPIX + qr * QPIX + j * CIN
                nc.tensor.transpose(tps[:, QCK + j:QCK + j + 1],
                                    s12row[0:1, o2:o2 + CIN], one32)

        def stats_finalize(qr):
            qsl = slice(qr * QPIX, (qr + 1) * QPIX)
            qsl2 = slice(NPIX + qr * QPIX, NPIX + (qr + 1) * QPIX)
            # pack quarter via PE transposes: tps[p, b*QCK+j] = s_b[qr*1024+j*128+p]
            tps = ps_s.tile([CIN, 2 * QCK], f32, tag="pss")
            transpose_pack(qr, tps, 0)
            yield
            transpose_pack(qr, tps, 1)
            yield
            S1 = tps[:, 0:QCK]
            S2 = tps[:, QCK:2 * QCK]
            stat2 = statp.tile([CIN, 3 * QCK], f32, tag=f"stat2{qr}")
            mean = stat2[:, 0:QCK]
            msq = stat2[:, QCK:2 * QCK]
            var = stat2[:, 2 * QCK:3 * QCK]
            nc.vector.tensor_scalar_mul(mean[:], S1[:], 1.0 / CIN)
            yield
            nc.vector.tensor_tensor(msq[:], mean[:], mean[:], mult)
            nc.vector.scalar_tensor_tensor(var[:], S2[:], 1.0 / CIN, msq[:], mult, sub)
            nc.vector.tensor_scalar_add(var[:], var[:], 1e-5)
            yield
            std = statp.tile([CIN, QCK], f32, tag=f"std{qr}")
            nc.scalar.sqrt(std[:], var[:])
            rstd32 = statp.tile([CIN, QCK], f32, tag=f"rstd32{qr}")
            nc.vector.reciprocal_approx_fast(rstd32[:], std[:])
            stat_bf = statp.tile([CIN, 2 * QCK], bf16, tag=f"stat_bf{qr}")
            nc.vector.tensor_copy(stat_bf[:, 0:QCK], rstd32[:])
            yield
            nc.vector.tensor_copy(stat_bf[:, QCK:2 * QCK], mean[:])
            yield
            # DMAs to DRAM rows; pixel index = qr*1024 + j*128 + p
            dd0 = s_dram[0:1, 0:1]
            for row, scols in ((0, slice(0, QCK)), (1, slice(QCK, 2 * QCK))):
                ddst = bass.AP(tensor=dd0.tensor,
                               offset=dd0.offset + row * NPIX + qr * QPIX,
                               ap=[[1, CIN], [CIN, QCK]])
                nc.sync.dma_start(ddst, stat_bf[:, scols])
                yield
            # partition-broadcast back into smb ([rstd | mu] column blocks)
            dd = s_dram[0:1, 0:1]
            for row, dcols in ((0, qsl), (1, qsl2)):
                src = bass.AP(tensor=dd.tensor,
                              offset=dd.offset + row * NPIX + qr * QPIX,
                              ap=[[0, CIN], [1, QPIX]])
                nc.sync.dma_start(smb[:, dcols], src)
                yield

        for pad_t in (k_pad, v_pad):
            nc.gpsimd.memset(pad_t[:, 0:PW + 1], 0.0)
            nc.gpsimd.memset(
                pad_t[:, PW + 65:PW + 65 + 64 * PW].rearrange(
                    "p (r t) -> p r t", t=PW)[:, :, 0:2], 0.0)
            nc.gpsimd.memset(pad_t[:, 65 * PW + 1:NPAD], 0.0)

        z_tiles = [None] * NCHUNK
        q_tiles = [None] * NCHUNK

        def pad_view(t, c, delta=0):
            off = (1 + c * ROWS_PER_CHUNK) * PW + 1 + delta
            return t[:, off:off + ROWS_PER_CHUNK * PW].rearrange(
                "p (r w) -> p r w", r=ROWS_PER_CHUNK, w=PW)[:, :, 0:W]

        def proj_gen(c):
            sl = slice(c * CHUNK, (c + 1) * CHUNK)
            tmp = tmpp.tile([CIN, CHUNK], bf16, tag="tmp")
            nc.vector.tensor_tensor(
                tmp[:], x_sb[:, sl],
                smb[:, NPIX + c * CHUNK:NPIX + (c + 1) * CHUNK], sub)
            yield
            z = zp.tile([CIN, CHUNK], bf16, tag="z")
            z_tiles[c] = z
            nc.vector.tensor_tensor(z[:], tmp[:], smb[:, sl], mult)
            yield
            qps = ps_a.tile([CIN, CHUNK], f32, tag="ps_a")
            mm(qps[:], wq, z[:], start=True, stop=True)
            yield
            q_c = qp_pool.tile([CIN, CHUNK], bf16, tag="q")
            q_tiles[c] = q_c
            nc.scalar.add(q_c[:], qps[:], bqc)                  # ACT
            yield
            kps = ps_a.tile([CIN, CHUNK], f32, tag="ps_a")
            mm(kps[:], wk, z[:], start=True, stop=True)
            yield
            nc.scalar.add(pad_view(k_pad, c)[:],
                          kps[:].rearrange("p (r w) -> p r w",
                                           r=ROWS_PER_CHUNK, w=W), bkc)  # ACT
            yield
            vps = ps_a.tile([CIN, CHUNK], f32, tag="ps_a")
            mm(vps[:], wv, z[:], start=True, stop=True)
            yield
            nc.scalar.copy(pad_view(v_pad, c)[:],
                           vps[:].rearrange("p (r w) -> p r w",
                                            r=ROWS_PER_CHUNK, w=W))  # ACT
            yield

        def scores_gen(c):
            q_v = q_tiles[c][:].rearrange("p (r w) -> p r w", r=ROWS_PER_CHUNK, w=W)
            sc = ps_s.tile([NH * K2, CHUNK], f32, tag="pss")
            mm(sc[:], wqs, z_tiles[c][:], start=True, stop=False)
            yield
            for k in range(K2):
                pk = pkp.tile([CIN, CHUNK], bf16, tag="pk")
                pk_v = pk[:].rearrange("p (r w) -> p r w", r=ROWS_PER_CHUNK, w=W)
                eng = nc.gpsimd if k >= 5 else nc.vector
                eng.tensor_tensor(pk_v[:], q_v[:],
                                  pad_view(k_pad, c, _shift_delta(k))[:], mult)
                yield
                mm(sc[:], cb16[:, 420 + k * NH * K2:420 + (k + 1) * NH * K2],
                   pk[:], start=False, stop=(k == K2 - 1))
                yield
            exp_c = smallp.tile([NH * K2, CHUNK], bf16, tag="exp")
            nc.scalar.activation(exp_c[:], sc[:], AF.Exp, bias=bqsc)  # ACT
            yield
            dn = ps_s.tile([NH * K2, CHUNK], f32, tag="pss")
            mm(dn[:], rsw, exp_c[:], start=True, stop=True)
            yield
            rcp = smallp.tile([NH * K2, CHUNK], f32, tag="rcp")
            nc.vector.reciprocal_approx_fast(rcp[:], dn[:])
            yield
            rcp_bf = smallp.tile([NH * K2, CHUNK], bf16, tag="rcpbf")
            nc.scalar.copy(rcp_bf[:], rcp[:])                       # ACT
            yield
            attn_c = smallp.tile([NH * K2, CHUNK], bf16, tag="attn")
            nc.vector.tensor_tensor(attn_c[:], exp_c[:], rcp_bf[:], mult)
            attn_tiles[c] = attn_c
            yield

        def av_gen(c, split=False):
            sl = slice(c * CHUNK, (c + 1) * CHUNK)
            attn_c = attn_tiles[c]
            if split:
                acc = ps_o.tile([COUT, CHUNK // 2], f32, tag="acc")
                accR = ps_a.tile([COUT, CHUNK // 2], f32, tag="ps_a")
            else:
                acc = ps_o.tile([COUT, CHUNK], f32, tag="acc")
            # rep matmuls write bf16 PSUM (exact: 0/1 matrix x bf16 attn);
            # mk multiplies read PSUM directly as 2-byte packed operands.
            # 'e': ACT-evict + DVE bf16 mult for a couple of ks to offload DVE.
            modes = ['e', 'd', 'e', 'd', 'e', 'd', 'e', 'd', 'e']
            for k in range(K2):
                rep = ps_r.tile([CIN, CHUNK], f32, tag="rep")
                mm(rep[:], cb16[0:NH * K2, 873 + k * CIN:873 + (k + 1) * CIN],
                   attn_c[:], start=True, stop=True)
                yield
                mk = mkp.tile([CIN, CHUNK], bf16, tag="mk")
                mk_v = mk[:].rearrange("p (r w) -> p r w", r=ROWS_PER_CHUNK, w=W)
                vv = pad_view(v_pad, c, _shift_delta(k))
                if modes[k] == 'd':
                    nc.vector.tensor_tensor(
                        mk_v[:], rep[:].rearrange("p (r w) -> p r w",
                                                  r=ROWS_PER_CHUNK, w=W),
                        vv[:], mult)
                    yield
                else:
                    rep_sb = repp.tile([CIN, CHUNK], bf16, tag="repsb")
                    nc.scalar.copy(rep_sb[:], rep[:])
                    yield
                    nc.vector.tensor_tensor(
                        mk_v[:], rep_sb[:].rearrange("p (r w) -> p r w",
                                                     r=ROWS_PER_CHUNK, w=W),
                        vv[:], mult)
                    yield
                if split:
                    HC = CHUNK // 2
                    mm(acc[:], wf, mk[:, 0:HC],
                       start=(k == 0), stop=(k == K2 - 1))
                    yield
                    mm(accR[:], wf, mk[:, HC:CHUNK],
                       start=(k == 0), stop=(k == K2 - 1))
                else:
                    mm(acc[:], wf, mk[:], start=(k == 0), stop=(k == K2 - 1))
                yield
            out_sb = outp.tile([COUT, CHUNK], f32, tag="outsb")
            if split:
                HC = CHUNK // 2
                nc.scalar.add(out_sb[:, 0:HC], acc[:], bfb)         # ACT
                yield
                nc.sync.dma_start(out_ext[:, c * CHUNK:c * CHUNK + HC],
                                  out_sb[:, 0:HC])
                yield
                nc.scalar.add(out_sb[:, HC:CHUNK], accR[:], bfb)
                yield
                nc.sync.dma_start(out_ext[:, c * CHUNK + HC:(c + 1) * CHUNK],
                                  out_sb[:, HC:CHUNK])
            else:
                nc.scalar.add(out_sb[:], acc[:], bfb)            # ACT
                yield
                nc.sync.dma_start(out_ext[:, sl], out_sb[:])
            yield

        def av_half(c, h):
            # 256-px half-chunk AV chain for the drain: all mults read rep
            # PSUM directly on DVE; two halves interleave to halve the tail.
            HP = CHUNK // 2
            hsl = slice(c * CHUNK + h * HP, c * CHUNK + (h + 1) * HP)
            attn_c = attn_tiles[c]
            if h == 0:
                acc = ps_o.tile([COUT, CHUNK // 2], f32, tag="acc")
            else:
                acc = ps_a.tile([COUT, CHUNK // 2], f32, tag="ps_a")
            for k in range(K2):
                rep = ps_r.tile([CIN, HP], f32, tag="rep")
                mm(rep[:], cb16[0:NH * K2, 873 + k * CIN:873 + (k + 1) * CIN],
                   attn_c[:, h * HP:(h + 1) * HP], start=True, stop=True)
                yield
                mk = mkp.tile([CIN, HP], bf16, tag="mk")
                off = (1 + c * ROWS_PER_CHUNK + h * 4) * PW + 1 + _shift_delta(k)
                vv = v_pad[:, off:off + 4 * PW].rearrange(
                    "p (r w) -> p r w", r=4, w=PW)[:, :, 0:W]
                nc.vector.tensor_tensor(
                    mk[:].rearrange("p (r w) -> p r w", r=4, w=W),
                    rep[:].rearrange("p (r w) -> p r w", r=4, w=W),
                    vv[:], mult)
                yield
                mm(acc[:], wf, mk[:], start=(k == 0), stop=(k == K2 - 1))
                yield
            out_sb = outp.tile([COUT, HP], f32, tag=f"outh{h}")
            nc.scalar.add(out_sb[:], acc[:], bfb)                # ACT
            yield
            nc.sync.dma_start(out_ext[:, hsl], out_sb[:])
            yield

        def run_all(gens):
            gens = [g for g in gens if g is not None]
            while gens:
                alive = []
                for g in gens:
                    try:
                        next(g)
                        alive.append(g)
                    except StopIteration:
                        pass
                gens = alive

        attn_tiles = [None] * NCHUNK
        for qr in range(3):
            run_all([stats_quarter(qr)])
        run_all([stats_quarter(3), proj_gen(0), proj_gen(1)])
        run_all([scores_gen(0), proj_gen(2)])
        # steady 2-deep software pipeline: SCORES(c) | AV(c-1) | PROJ(c+2)
        for c in range(1, NCHUNK):
            run_all([scores_gen(c), av_gen(c - 1),
                     proj_gen(c + 2) if c + 2 < NCHUNK else None])
        run_all([av_gen(NCHUNK - 1, split=True)])


def _get_compiled():
    if "nc" not in _CACHE:
        _CACHE["nc"] = _build_bass()
    return _CACHE["nc"]


def kernel(**inputs):
    x = np.asarray(inputs["x"], dtype=np.float32)          # [B, CIN, H, W]
    consts = _host_fold(
        np.asarray(inputs["ln_g"]), np.asarray(inputs["ln_b"]),
        np.asarray(inputs["Wq"]), np.asarray(inputs["bq"]),
        np.asarray(inputs["Wk"]), np.asarray(inputs["bk"]),
        np.asarray(inputs["Wv"]), np.asarray(inputs["bv"]),
        np.asarray(inputs["Wp"]), np.asarray(inputs["bp"]),
        np.asarray(inputs["Wf"]), np.asarray(inputs["bf"]),
    )

    nc = _get_compiled()

    from concourse.bass_utils import run_bass_kernel_spmd

    core_ids = list(range(B))
    in_maps = []
    for i in range(B):
        m = {"x": np.ascontiguousarray(x[i].reshape(CIN, NPIX))}
        m.update(consts)
        in_maps.append(m)

    res = run_bass_kernel_spmd(nc, in_maps, core_ids,
                               trace=bool(int(os.environ.get("KTRACE", "0"))))
    _CACHE["last_result"] = res
    out = np.stack([res.results[i]["out"].reshape(COUT, H, W) for i in range(B)])
    return out.astype(np.float32)


if __name__ == "__main__":
    nc = _get_compiled()
    print("compiled OK")
